# revision 29
# baseline (speedup 1.0000x reference)
"""AddLinearAttention Trainium2 kernel — 8-core data-parallel over batch.

B=16, C=128, H=W=96. Each of 8 cores handles 2 batches, channel-major
(C=128 partitions, HW free). Depthwise convs run as fp8 DoubleRow
pair-matmuls over zero-padded fp8 images (two vertically-adjacent taps
per PE instruction at 0.5 cyc/row); the center tap + residual ride a
row-scaled copy of the 1x1-conv weights written strided into the same
PSUM accumulation, so elu() reads conv+residual straight from PSUM.
"""

import os
from contextlib import ExitStack

import numpy as np
import ml_dtypes

import concourse.bass as bass
import concourse.tile as tile
from concourse import bacc, mybir
from concourse._compat import with_exitstack
from concourse.bass_utils import run_bass_kernel_spmd

B, C, H, W = 16, 128, 96, 96
HW = H * W
NCORES = 8
BPC = B // NCORES          # batches per core
RPC = 4                    # image rows per outer chunk
NCH = H // RPC             # chunks per image (24)
PAD = 2
WP = W + 2 * PAD           # 100
HP = H + 2 * PAD + 3       # 103: tail rows keep flat tap runs in-bounds
SCALE = C ** (-0.5)
S2 = SCALE / HW            # kv scale
ZS = SCALE / HW            # z scale folds k_mean's 1/HW and the C^-0.5
WSC = 256.0                # fp8 tap-weight scale (power of 2)

F32 = mybir.dt.float32
BF16 = mybir.dt.bfloat16
FP8 = mybir.dt.float8e4
AF = mybir.ActivationFunctionType
ALU = mybir.AluOpType
PM = mybir.MatmulPerfMode

DX3 = (-1, 0, 1)
DX5 = (-2, -1, 0, 1, 2)
# vertical DoubleRow pairs ((dy_a,dx),(dy_a+2,dx)); taps with no vertical
# partner get a zero second diagonal (center (0,0) rides wqkvoc instead)
P3 = [(-1, dx) for dx in DX3] + [(0, -1), (0, 1)]                  # 5
P5 = [(-2, dx) for dx in DX5] + [(-1, dx) for dx in DX5] + [(2, dx) for dx in DX5]  # 15
NPR = 3 * len(P3) + len(P5)                        # 30


@with_exitstack
def _build(ctx: ExitStack, tc: tile.TileContext, zb: bool):
    nc = tc.nc
    x_d = nc.dram_tensor("x", [BPC, C, H, W], BF16, kind="ExternalInput").ap()
    wqkvo_d = nc.dram_tensor("wqkvo_t", [C, 4 * C], F32, kind="ExternalInput").ap()
    wqkvoc_d = nc.dram_tensor("wqkvoc_t", [C, 3 * C], F32, kind="ExternalInput").ap()
    wproj_d = nc.dram_tensor("wproj_t", [C, C], F32, kind="ExternalInput").ap()
    pairs_d = nc.dram_tensor("dwpairs", [C, NPR, 2, C], FP8, kind="ExternalInput").ap()
    bias_d = nc.dram_tensor("biases", [C, 9], F32, kind="ExternalInput").ap()
    ident_d = nc.dram_tensor("ident", [C, C], F32, kind="ExternalInput").ap()
    out_d = nc.dram_tensor("out", [BPC, C, H, W], BF16, kind="ExternalOutput").ap()

    const = ctx.enter_context(tc.tile_pool(name="const", bufs=1))
    wq_sb = const.tile([C, 4 * C], BF16, tag="wq")
    wqc_sb = const.tile([C, 3 * C], BF16, tag="wqc")
    wp_sb = const.tile([C, C], BF16, tag="wp")
    pr_sb = const.tile([C, NPR, 2, C], FP8, tag="pr")
    bi_sb = const.tile([C, 9], F32, tag="bi")
    id_sb = const.tile([C, C], BF16, tag="id")
    ones_sb = const.tile([C, C], BF16, tag="ones")
    inv_sb = const.tile([C, C], BF16, tag="inv")
    kmr_sb = const.tile([C, C], BF16, tag="kmr")
    ks_sb = const.tile([C, 1], F32, tag="ks")

    # f32 -> bf16 casts ride the SWDGE DMA; fp8 ships raw
    nc.gpsimd.dma_start(out=wq_sb[:], in_=wqkvo_d[:])
    nc.gpsimd.dma_start(out=wqc_sb[:], in_=wqkvoc_d[:])
    nc.gpsimd.dma_start(out=wp_sb[:], in_=wproj_d[:])
    nc.sync.dma_start(out=pr_sb[:], in_=pairs_d[:])
    nc.sync.dma_start(out=bi_sb[:], in_=bias_d[:])
    nc.gpsimd.dma_start(out=id_sb[:], in_=ident_d[:])
    nc.vector.memset(ones_sb[:], 1.0)
    nc.vector.memset(inv_sb[:], 1.0 / C)

    bq, bk, bv, bo = (bi_sb[:, i : i + 1] for i in range(4))
    beq, bek, bev, blep, bprj = (bi_sb[:, i : i + 1] for i in range(4, 9))

    pads = ctx.enter_context(tc.tile_pool(name="pads", bufs=1))
    qpad = pads.tile([C, HP, WP], FP8, tag="qpad")
    kpad = pads.tile([C, HP, WP], FP8, tag="kpad")
    vpad = pads.tile([C, HP, WP], FP8, tag="vpad")
    for t in (qpad, kpad, vpad):
        # interior is rewritten every batch; zero only the border cells
        nc.vector.memset(t[:, :PAD, :], 0.0)                  # top rows
        nc.vector.memset(t[:, PAD + H :, :], 0.0)             # bottom + tail
        nc.vector.memset(t[:, PAD : PAD + H, :PAD], 0.0)      # left cols
        nc.vector.memset(t[:, PAD : PAD + H, PAD + W :], 0.0)  # right cols

    APc = type(qpad[:])

    def pad_pair(pad, dy_a, dx, y0, s):
        f = pad[:]
        off = (PAD + y0 + 2 * s + dy_a) * WP + (PAD + dx)
        return APc(f.tensor, f.offset + off, [[f.ap[0][0], C], [2 * WP, 2], [1, 2 * WP]])

    def pad_run(pad, dy, dx, y0, s):
        f = pad[:]
        off = (PAD + y0 + 2 * s + dy) * WP + (PAD + dx)
        return APc(f.tensor, f.offset + off, [[f.ap[0][0], C], [1, 2 * WP]])

    def ps_run(pt, s):
        f = pt[:]
        return APc(f.tensor, f.offset + s * 2 * WP, [[f.ap[0][0], C], [1, 2 * WP]])

    def ps_img(pt):
        f = pt[:]
        return APc(f.tensor, f.offset, [[f.ap[0][0], C], [WP, RPC], [1, W]])

    flats = ctx.enter_context(tc.tile_pool(name="flats", bufs=1))
    xin = ctx.enter_context(tc.tile_pool(name="xin", bufs=2))
    etmp = ctx.enter_context(tc.tile_pool(name="etmp", bufs=4))
    ctmp = ctx.enter_context(tc.tile_pool(name="ctmp", bufs=2))
    ttmp = ctx.enter_context(tc.tile_pool(name="ttmp", bufs=6))
    ostg = ctx.enter_context(tc.tile_pool(name="ostg", bufs=2))
    kvs = ctx.enter_context(tc.tile_pool(name="kvs", bufs=2))

    for b in range(BPC):
        x_sb = xin.tile([C, H, W], BF16, tag="x")
        o_sb = flats.tile([C, H, W], BF16, tag="o")
        qh = flats.tile([C, H, W], BF16, tag="qh")
        kh = flats.tile([C, H, W], BF16, tag="kh")
        vh = flats.tile([C, H, W], BF16, tag="vh")
        lep = flats.tile([C, H, W], BF16, tag="lep")
        res = flats.tile([C, H, W], BF16, tag="kh")    # kh dead after kv phase
        ks24 = flats.tile([C, NCH], F32, tag="ks24")   # per-chunk kh row sums
        nc.sync.dma_start(out=x_sb[:], in_=x_d[b])

        # ---- phase A: qkvo 1x1 (lead 2) + fp8 DoubleRow depthwise convs
        with tc.tile_pool(name="ps_a", bufs=2, space="PSUM") as psa, \
             tc.tile_pool(name="ps_qk", bufs=2, space="PSUM") as pqk, \
             tc.tile_pool(name="ps_vl", bufs=1, space="PSUM") as pvl:
          for rr in range(NCH + 2):
            # qkvo matmuls + evacs for chunk rr, interleaved with taps of
            # chunk rr-2 so PE never stalls on the rotating qkvo banks
            def qkvo_mm(g):
                y0n = rr * RPC
                p = psa.tile([C, RPC, W], F32, tag="pg")
                nc.tensor.matmul(
                    p[:], wq_sb[:, g * C : (g + 1) * C],
                    x_sb[:, y0n : y0n + RPC, :], start=True, stop=True,
                )
                return p

            def qkvo_evac(g, p):
                y0n = rr * RPC
                if g < 3:
                    pad = (qpad, kpad, vpad)[g]
                    dstw = pad[:, PAD + y0n : PAD + y0n + RPC, PAD : PAD + W]
                    bb = (bq, bk, bv)[g]
                    if g == 0:
                        nc.vector.tensor_scalar_add(dstw, p[:], bb)
                    else:
                        nc.scalar.activation(dstw, p[:], AF.Identity, bias=bb)
                else:
                    nc.scalar.activation(
                        o_sb[:, y0n : y0n + RPC, :], p[:], AF.Identity, bias=bo
                    )

            do_qkvo = rr < NCH
            do_taps = rr >= 2
            r = rr - 2
            y0 = r * RPC

            if do_qkvo:
                pq_ = qkvo_mm(0)
                pk_ = qkvo_mm(1)
                qkvo_evac(0, pq_)
                qkvo_evac(1, pk_)

            def tap_group(pool, tag, pr0, taps, pad, center_col):
                # PSUM bank rule: a second start=True closes the prior group,
                # so each sub-group OPENS with its own center half (start=True)
                # and the DoubleRow pairs accumulate behind it.
                ct = pool.tile([C, RPC, WP], F32, tag=tag)
                ctf = ct[:]
                for s in (0, 1):
                    if center_col is not None:
                        cout = APc(ctf.tensor, ctf.offset + s * 2 * WP,
                                   [[ctf.ap[0][0], C], [WP, 2], [1, W]])
                        nc.tensor.matmul(
                            cout, wqc_sb[:, center_col * C : (center_col + 1) * C],
                            x_sb[:, y0 + 2 * s : y0 + 2 * s + 2, :],
                            start=True, stop=False, skip_group_check=True,
                        )
                    for i, (dy, dx) in enumerate(taps):
                        nc.tensor.matmul(
                            ps_run(ct, s), pr_sb[:, pr0 + i],
                            pad_pair(pad, dy, dx, y0, s),
                            start=(center_col is None and i == 0),
                            stop=(s == 1 and i == len(taps) - 1),
                            perf_mode=PM.DoubleRow, skip_group_check=True,
                        )
                return ct

            cq = ck = cv = cl = None
            if do_taps:
                cq = tap_group(pqk, "cq", 0, P3, qpad, 0)
                ck = tap_group(pqk, "ck", 5, P3, kpad, 1)
            if do_qkvo:
                pv_ = qkvo_mm(2)
                po_ = qkvo_mm(3)
                qkvo_evac(2, pv_)
                qkvo_evac(3, po_)
            if do_taps:
                cv = tap_group(pvl, "cv", 10, P3, vpad, 2)
                cl = tap_group(pvl, "cl", 15, P5, vpad, None)

                # elu(s)+1 = min(exp(s),1) + max(s,0), s read straight from PSUM
                for ct, beff, dst, acc in ((cq, beq, qh, None),
                                           (ck, bek, kh, True)):
                    e = etmp.tile([C, RPC, W], BF16, tag="e")
                    nc.scalar.activation(
                        e[:], ps_img(ct), AF.Exp, bias=beff, scale=1.0 / WSC
                    )
                    rl = etmp.tile([C, RPC, W], BF16, tag="rl")
                    if zb:
                        nc.vector.tensor_scalar(
                            rl[:], ps_img(ct), 1.0 / WSC, 0.0, ALU.mult, ALU.max
                        )
                    else:
                        rs = etmp.tile([C, RPC, W], BF16, tag="rs")
                        nc.vector.tensor_scalar(
                            rs[:], ps_img(ct), 1.0 / WSC, beff, ALU.mult, ALU.add
                        )
                        nc.vector.tensor_scalar_max(rl[:], rs[:], 0.0)
                    nc.vector.scalar_tensor_tensor(
                        dst[:, y0 : y0 + RPC, :], e[:], 1.0, rl[:], ALU.min, ALU.add,
                        accum_out=ks24[:, r : r + 1] if acc else None,
                    )
                nc.vector.tensor_scalar(
                    vh[:, y0 : y0 + RPC, :], ps_img(cv), 1.0 / WSC, bev,
                    ALU.mult, ALU.add,
                )
                nc.scalar.activation(
                    lep[:, y0 : y0 + RPC, :], ps_img(cl), AF.Identity,
                    bias=blep, scale=1.0 / WSC,
                )

        # ---- k_mean from the per-chunk accum_out sums -> replicated lhsT
        nc.vector.tensor_reduce(ks_sb[:], ks24[:], axis=mybir.AxisListType.X, op=ALU.add)
        nc.vector.tensor_scalar(
            kmr_sb[:], ones_sb[:], ks_sb[:], ZS, ALU.mult, ALU.mult
        )

        # ---- kv = s2 * k~^T v~ via PE transposes, 72 token chunks;
        # kt/vt share one PSUM bank, one evac copy alternating DVE/Act
        kv_sb = kvs.tile([C, C], BF16, tag="kv")
        with tc.tile_pool(name="kvp", bufs=1, space="PSUM") as kvp, \
             tc.tile_pool(name="tpp", bufs=6, space="PSUM") as tpp:
          kvacc = kvp.tile([C, C], F32, tag="kvacc")
          kf = kh[:].rearrange("p a b -> p (a b)")
          vf = vh[:].rearrange("p a b -> p (a b)")
          for j in range(HW // C):
            tp = tpp.tile([C, 2 * C], BF16, tag="tp")
            nc.tensor.transpose(tp[:, :C], kf[:, j * C : (j + 1) * C], id_sb[:])
            nc.tensor.transpose(tp[:, C:], vf[:, j * C : (j + 1) * C], id_sb[:])
            kvt = ttmp.tile([C, 2 * C], BF16, tag="kvt")
            if j % 2 == 0:
                nc.vector.tensor_copy(kvt[:], tp[:])
            else:
                nc.scalar.activation(kvt[:], tp[:], AF.Copy)
            nc.tensor.matmul(
                kvacc[:], kvt[:, :C], kvt[:, C:],
                start=(j == 0), stop=(j == HW // C - 1),
            )
          nc.scalar.activation(kv_sb[:], kvacc[:], AF.Copy, scale=float(S2))

        # ---- res = (q~ @ kv)(1 + 1/z) - z*vbar ; + lepe ; * o ; proj ; out
        with tc.tile_pool(name="ps_e", bufs=2, space="PSUM") as ps, \
             tc.tile_pool(name="ps_f", bufs=2, space="PSUM") as psf:
          for r in range(NCH + 2):
            if r < NCH:
                y0 = r * RPC
                rq = qh[:, y0 : y0 + RPC, :]
                rv = vh[:, y0 : y0 + RPC, :]
                zp = ps.tile([C, RPC, W], F32, tag="zp")
                nc.tensor.matmul(zp[:], kmr_sb[:], rq, start=True, stop=True)
                vb = ps.tile([C, RPC, W], F32, tag="vb")
                nc.tensor.matmul(vb[:], inv_sb[:], rv, start=True, stop=True)
                rp = ps.tile([C, RPC, W], F32, tag="rp")
                nc.tensor.matmul(rp[:], kv_sb[:], rq, start=True, stop=True)
            if r >= 2:
                yp = (r - 2) * RPC
                pp = psf.tile([C, RPC, W], F32, tag="pp")
                nc.tensor.matmul(
                    pp[:], wp_sb[:], res[:, yp : yp + RPC, :], start=True, stop=True
                )
                og = ostg.tile([C, RPC, W], BF16, tag="og")
                nc.scalar.activation(og[:], pp[:], AF.Identity, bias=bprj)
                nc.sync.dma_start(out=out_d[b, :, yp : yp + RPC, :], in_=og[:])
            if r < NCH:
                rr_t = ctmp.tile([C, RPC, W], BF16, tag="rr")
                nc.vector.reciprocal(rr_t[:], zp[:])
                t1 = ctmp.tile([C, RPC, W], BF16, tag="t1")
                nc.vector.scalar_tensor_tensor(
                    t1[:], rr_t[:], 1.0, rp[:], ALU.add, ALU.mult
                )
                vbs = ctmp.tile([C, RPC, W], BF16, tag="vbs")
                nc.scalar.activation(vbs[:], vb[:], AF.Copy)
                t2 = ctmp.tile([C, RPC, W], BF16, tag="t2")
                nc.vector.tensor_tensor(t2[:], zp[:], vbs[:], ALU.mult)
                t3 = ctmp.tile([C, RPC, W], BF16, tag="t3")
                nc.gpsimd.tensor_tensor(t3[:], t1[:], t2[:], ALU.subtract)
                t4 = ctmp.tile([C, RPC, W], BF16, tag="t4")
                nc.gpsimd.tensor_tensor(t4[:], t3[:], lep[:, y0 : y0 + RPC, :], ALU.add)
                nc.vector.tensor_tensor(
                    res[:, y0 : y0 + RPC, :], t4[:], o_sb[:, y0 : y0 + RPC, :], ALU.mult
                )


_CACHE = {}


def _get_nc(zb: bool = True):
    key = ("nc", zb)
    if key not in _CACHE:
        nc = bacc.Bacc("TRN2", target_bir_lowering=False, debug=False)
        with tile.TileContext(nc, pool_alloc_mode="queue") as tc, \
             nc.allow_low_precision(reason="bf16/fp8 compute pipeline"):
            _build(tc, zb)
        nc.compile()
        _CACHE[key] = nc
    return _CACHE[key]


def _diag(v):
    d = np.zeros((C, C), np.float32)
    np.fill_diagonal(d, v)
    return d


def kernel(**inputs) -> np.ndarray:
    x = np.asarray(inputs["x"], np.float32)
    w_qkvo = np.asarray(inputs["w_qkvo"], np.float32)[:, :, 0, 0]  # (4C, C)
    b_qkvo = np.asarray(inputs["b_qkvo"], np.float32)
    w_lepe = np.asarray(inputs["w_lepe"], np.float32)[:, 0]        # (C,5,5)
    b_lepe = np.asarray(inputs["b_lepe"], np.float32)
    w_proj = np.asarray(inputs["w_proj"], np.float32)[:, :, 0, 0]
    b_proj = np.asarray(inputs["b_proj"], np.float32)
    w_q = np.asarray(inputs["w_q"], np.float32)[:, 0]              # (C,3,3)
    b_q = np.asarray(inputs["b_q"], np.float32)
    w_k = np.asarray(inputs["w_k"], np.float32)[:, 0]
    b_k = np.asarray(inputs["b_k"], np.float32)
    w_v = np.asarray(inputs["w_v"], np.float32)[:, 0]
    b_v = np.asarray(inputs["b_v"], np.float32)

    fp8 = ml_dtypes.float8_e4m3
    bf16 = ml_dtypes.bfloat16

    # DoubleRow pair diagonals (w * WSC); taps without a vertical partner
    # get a zero second diagonal
    pairs = np.zeros((NPR, 2, C, C), np.float32)
    for t, wt in enumerate((w_q, w_k, w_v)):
        for i, (dy, dx) in enumerate(P3):
            pairs[5 * t + i, 0] = _diag(wt[:, dy + 1, dx + 1] * WSC)
            if dy + 3 <= 2:
                pairs[5 * t + i, 1] = _diag(wt[:, dy + 3, dx + 1] * WSC)
    for i, (dy, dx) in enumerate(P5):
        pairs[15 + i, 0] = _diag(w_lepe[:, dy + 2, dx + 2] * WSC)
        if dy + 4 <= 4:
            pairs[15 + i, 1] = _diag(w_lepe[:, dy + 4, dx + 2] * WSC)
    dwpairs = np.ascontiguousarray(pairs.transpose(2, 0, 1, 3)).astype(fp8)

    # center+residual columns: Wg row-scaled by (1+w_center)*WSC, transposed
    wqkvoc = np.concatenate(
        [
            (w_qkvo[t * C : (t + 1) * C] * ((1.0 + wt[:, 1, 1]) * WSC)[:, None]).T
            for t, wt in enumerate((w_q, w_k, w_v))
        ],
        axis=1,
    )  # (C, 3C)

    beff_q = b_q + (1.0 + w_q[:, 1, 1]) * b_qkvo[:C]
    beff_k = b_k + (1.0 + w_k[:, 1, 1]) * b_qkvo[C : 2 * C]
    beff_v = b_v + (1.0 + w_v[:, 1, 1]) * b_qkvo[2 * C : 3 * C]
    biases = np.stack(
        [
            b_qkvo[:C], b_qkvo[C : 2 * C], b_qkvo[2 * C : 3 * C], b_qkvo[3 * C :],
            beff_q, beff_k, beff_v, b_lepe, b_proj,
        ],
        axis=1,
    ).astype(np.float32)
    zb = not (np.any(beff_q) or np.any(beff_k))

    shared = {
        "wqkvo_t": np.ascontiguousarray(w_qkvo.T),
        "wqkvoc_t": np.ascontiguousarray(wqkvoc),
        "wproj_t": np.ascontiguousarray(w_proj.T),
        "dwpairs": dwpairs,
        "biases": biases,
        "ident": np.eye(C, dtype=np.float32),
    }
    xb = x.astype(bf16).reshape(NCORES, BPC, C, H, W)
    in_maps = [{"x": np.ascontiguousarray(xb[i]), **shared} for i in range(NCORES)]

    nc = _get_nc(zb)
    _CACHE["last_in_maps"] = in_maps
    r = run_bass_kernel_spmd(
        nc, in_maps, core_ids=list(range(NCORES)),
        trace=bool(int(os.environ.get("KERNEL_TRACE", "0"))),
    )
    _CACHE["last_results"] = r
    out = np.stack([np.asarray(r.results[i]["out"]) for i in range(NCORES)])
    return out.astype(np.float32).reshape(B, C, H, W)


# revision 31
# speedup vs baseline: 1.0153x; 1.0153x over previous
"""AddLinearAttention Trainium2 kernel — 8-core data-parallel over batch.

B=16, C=128, H=W=96. Each of 8 cores handles 2 batches, channel-major
(C=128 partitions, HW free). Depthwise convs run as fp8 DoubleRow
pair-matmuls over zero-padded fp8 images (two vertically-adjacent taps
per PE instruction at 0.5 cyc/row); the center tap + residual ride a
row-scaled copy of the 1x1-conv weights written strided into the same
PSUM accumulation, so elu() reads conv+residual straight from PSUM.
"""

import os
from contextlib import ExitStack

import numpy as np
import ml_dtypes

import concourse.bass as bass
import concourse.tile as tile
from concourse import bacc, mybir
from concourse._compat import with_exitstack
from concourse.bass_utils import run_bass_kernel_spmd

B, C, H, W = 16, 128, 96, 96
HW = H * W
NCORES = 8
BPC = B // NCORES          # batches per core
RPC = 4                    # image rows per outer chunk
NCH = H // RPC             # chunks per image (24)
PAD = 2
WP = W + 2 * PAD           # 100
HP = H + 2 * PAD + 3       # 103: tail rows keep flat tap runs in-bounds
SCALE = C ** (-0.5)
S2 = SCALE / HW            # kv scale
ZS = SCALE / HW            # z scale folds k_mean's 1/HW and the C^-0.5
WSC = 256.0                # fp8 tap-weight scale (power of 2)

F32 = mybir.dt.float32
BF16 = mybir.dt.bfloat16
FP8 = mybir.dt.float8e4
AF = mybir.ActivationFunctionType
ALU = mybir.AluOpType
PM = mybir.MatmulPerfMode

DX3 = (-1, 0, 1)
DX5 = (-2, -1, 0, 1, 2)
# vertical DoubleRow pairs ((dy_a,dx),(dy_a+2,dx)); taps with no vertical
# partner get a zero second diagonal (center (0,0) rides wqkvoc instead)
P3 = [(-1, dx) for dx in DX3] + [(0, -1), (0, 1)]                  # 5
P5 = [(-2, dx) for dx in DX5] + [(-1, dx) for dx in DX5] + [(2, dx) for dx in DX5]  # 15
NPR = 3 * len(P3) + len(P5)                        # 30


@with_exitstack
def _build(ctx: ExitStack, tc: tile.TileContext, zb: bool):
    nc = tc.nc
    x_d = nc.dram_tensor("x", [BPC, C, H, W], BF16, kind="ExternalInput").ap()
    wqkvo_d = nc.dram_tensor("wqkvo_t", [C, 4 * C], F32, kind="ExternalInput").ap()
    wqkvoc_d = nc.dram_tensor("wqkvoc_t", [C, 3 * C], F32, kind="ExternalInput").ap()
    wproj_d = nc.dram_tensor("wproj_t", [C, C], F32, kind="ExternalInput").ap()
    pairs_d = nc.dram_tensor("dwpairs", [C, NPR, 2, C], FP8, kind="ExternalInput").ap()
    bias_d = nc.dram_tensor("biases", [C, 9], F32, kind="ExternalInput").ap()
    ident_d = nc.dram_tensor("ident", [C, C], F32, kind="ExternalInput").ap()
    out_d = nc.dram_tensor("out", [BPC, C, H, W], BF16, kind="ExternalOutput").ap()

    const = ctx.enter_context(tc.tile_pool(name="const", bufs=1))
    wq_sb = const.tile([C, 4 * C], BF16, tag="wq")
    wqc_sb = const.tile([C, 3 * C], BF16, tag="wqc")
    wp_sb = const.tile([C, C], BF16, tag="wp")
    pr_sb = const.tile([C, NPR, 2, C], FP8, tag="pr")
    bi_sb = const.tile([C, 9], F32, tag="bi")
    id_sb = const.tile([C, C], BF16, tag="id")
    ones_sb = const.tile([C, C], BF16, tag="ones")
    inv_sb = const.tile([C, C], BF16, tag="inv")
    kmr_sb = const.tile([C, C], BF16, tag="kmr")
    ks_sb = const.tile([C, 1], F32, tag="ks")

    # f32 -> bf16 casts ride the SWDGE DMA; fp8 ships raw
    nc.gpsimd.dma_start(out=wq_sb[:], in_=wqkvo_d[:])
    nc.gpsimd.dma_start(out=wqc_sb[:], in_=wqkvoc_d[:])
    nc.gpsimd.dma_start(out=wp_sb[:], in_=wproj_d[:])
    nc.sync.dma_start(out=pr_sb[:], in_=pairs_d[:])
    nc.sync.dma_start(out=bi_sb[:], in_=bias_d[:])
    nc.gpsimd.dma_start(out=id_sb[:], in_=ident_d[:])
    nc.vector.memset(ones_sb[:], 1.0)
    nc.vector.memset(inv_sb[:], 1.0 / C)

    bq, bk, bv, bo = (bi_sb[:, i : i + 1] for i in range(4))
    beq, bek, bev, blep, bprj = (bi_sb[:, i : i + 1] for i in range(4, 9))

    pads = ctx.enter_context(tc.tile_pool(name="pads", bufs=1))
    qpad = pads.tile([C, HP, WP], FP8, tag="qpad")
    kpad = pads.tile([C, HP, WP], FP8, tag="kpad")
    vpad = pads.tile([C, HP, WP], FP8, tag="vpad")
    for t in (qpad, kpad, vpad):
        # interior is rewritten every batch; zero only the border cells
        nc.vector.memset(t[:, :PAD, :], 0.0)                  # top rows
        nc.vector.memset(t[:, PAD + H :, :], 0.0)             # bottom + tail
        nc.vector.memset(t[:, PAD : PAD + H, :PAD], 0.0)      # left cols
        nc.vector.memset(t[:, PAD : PAD + H, PAD + W :], 0.0)  # right cols

    APc = type(qpad[:])

    def pad_pair(pad, dy_a, dx, y0, s):
        f = pad[:]
        off = (PAD + y0 + 2 * s + dy_a) * WP + (PAD + dx)
        return APc(f.tensor, f.offset + off, [[f.ap[0][0], C], [2 * WP, 2], [1, 2 * WP]])

    def pad_run(pad, dy, dx, y0, s):
        f = pad[:]
        off = (PAD + y0 + 2 * s + dy) * WP + (PAD + dx)
        return APc(f.tensor, f.offset + off, [[f.ap[0][0], C], [1, 2 * WP]])

    def ps_run(pt, s):
        f = pt[:]
        return APc(f.tensor, f.offset + s * 2 * WP, [[f.ap[0][0], C], [1, 2 * WP]])

    def ps_img(pt):
        f = pt[:]
        return APc(f.tensor, f.offset, [[f.ap[0][0], C], [WP, RPC], [1, W]])

    flats = ctx.enter_context(tc.tile_pool(name="flats", bufs=1))
    xin = ctx.enter_context(tc.tile_pool(name="xin", bufs=2))
    etmp = ctx.enter_context(tc.tile_pool(name="etmp", bufs=4))
    ctmp = ctx.enter_context(tc.tile_pool(name="ctmp", bufs=2))
    ttmp = ctx.enter_context(tc.tile_pool(name="ttmp", bufs=6))
    ostg = ctx.enter_context(tc.tile_pool(name="ostg", bufs=2))
    kvs = ctx.enter_context(tc.tile_pool(name="kvs", bufs=2))

    for b in range(BPC):
        x_sb = xin.tile([C, H, W], BF16, tag="x")
        o_sb = flats.tile([C, H, W], BF16, tag="o")
        qh = flats.tile([C, H, W], BF16, tag="qh")
        kh = flats.tile([C, H, W], BF16, tag="kh")
        vh = flats.tile([C, H, W], BF16, tag="vh")
        lep = flats.tile([C, H, W], BF16, tag="lep")
        res = flats.tile([C, H, W], BF16, tag="kh")    # kh dead after kv phase
        ks24 = flats.tile([C, NCH], F32, tag="ks24")   # per-chunk kh row sums
        nc.sync.dma_start(out=x_sb[:], in_=x_d[b])

        # ---- phase A: qkvo 1x1 (lead 2) + fp8 DoubleRow depthwise convs
        with tc.tile_pool(name="ps_a", bufs=2, space="PSUM") as psa, \
             tc.tile_pool(name="ps_qk", bufs=2, space="PSUM") as pqk, \
             tc.tile_pool(name="ps_vl", bufs=1, space="PSUM") as pvl:
          for rr in range(NCH + 2):
            # qkvo matmuls + evacs for chunk rr, interleaved with taps of
            # chunk rr-2 so PE never stalls on the rotating qkvo banks
            def qkvo_mm(g):
                y0n = rr * RPC
                p = psa.tile([C, RPC, W], F32, tag="pg")
                nc.tensor.matmul(
                    p[:], wq_sb[:, g * C : (g + 1) * C],
                    x_sb[:, y0n : y0n + RPC, :], start=True, stop=True,
                )
                return p

            def qkvo_evac(g, p):
                y0n = rr * RPC
                if g < 3:
                    pad = (qpad, kpad, vpad)[g]
                    dstw = pad[:, PAD + y0n : PAD + y0n + RPC, PAD : PAD + W]
                    bb = (bq, bk, bv)[g]
                    if g == 0:
                        nc.vector.tensor_scalar_add(dstw, p[:], bb)
                    else:
                        nc.scalar.activation(dstw, p[:], AF.Identity, bias=bb)
                else:
                    nc.scalar.activation(
                        o_sb[:, y0n : y0n + RPC, :], p[:], AF.Identity, bias=bo
                    )

            do_qkvo = rr < NCH
            do_taps = rr >= 2
            r = rr - 2
            y0 = r * RPC

            if do_qkvo:
                pq_ = qkvo_mm(0)
                pk_ = qkvo_mm(1)
                qkvo_evac(0, pq_)
                qkvo_evac(1, pk_)

            def tap_group(pool, tag, pr0, taps, pad, center_col):
                # PSUM bank rule: a second start=True closes the prior group,
                # so each sub-group OPENS with its own center half (start=True)
                # and the DoubleRow pairs accumulate behind it.
                ct = pool.tile([C, RPC, WP], F32, tag=tag)
                ctf = ct[:]
                for s in (0, 1):
                    if center_col is not None:
                        cout = APc(ctf.tensor, ctf.offset + s * 2 * WP,
                                   [[ctf.ap[0][0], C], [WP, 2], [1, W]])
                        nc.tensor.matmul(
                            cout, wqc_sb[:, center_col * C : (center_col + 1) * C],
                            x_sb[:, y0 + 2 * s : y0 + 2 * s + 2, :],
                            start=True, stop=False, skip_group_check=True,
                        )
                    for i, (dy, dx) in enumerate(taps):
                        nc.tensor.matmul(
                            ps_run(ct, s), pr_sb[:, pr0 + i],
                            pad_pair(pad, dy, dx, y0, s),
                            start=(center_col is None and i == 0),
                            stop=(s == 1 and i == len(taps) - 1),
                            perf_mode=PM.DoubleRow, skip_group_check=True,
                        )
                return ct

            cq = ck = cv = cl = None
            if do_taps:
                cq = tap_group(pqk, "cq", 0, P3, qpad, 0)
                ck = tap_group(pqk, "ck", 5, P3, kpad, 1)
            if do_qkvo:
                pv_ = qkvo_mm(2)
                po_ = qkvo_mm(3)
                qkvo_evac(2, pv_)
                qkvo_evac(3, po_)
            if do_taps:
                cv = tap_group(pvl, "cv", 10, P3, vpad, 2)
                cl = tap_group(pvl, "cl", 15, P5, vpad, None)

                # elu(s)+1 = min(exp(s),1) + max(s,0), s read straight from PSUM
                for ct, beff, dst, acc in ((cq, beq, qh, None),
                                           (ck, bek, kh, True)):
                    e = etmp.tile([C, RPC, W], BF16, tag="e")
                    nc.scalar.activation(
                        e[:], ps_img(ct), AF.Exp, bias=beff, scale=1.0 / WSC
                    )
                    rl = etmp.tile([C, RPC, W], BF16, tag="rl")
                    if zb:
                        nc.vector.tensor_scalar(
                            rl[:], ps_img(ct), 1.0 / WSC, 0.0, ALU.mult, ALU.max
                        )
                    else:
                        rs = etmp.tile([C, RPC, W], BF16, tag="rs")
                        nc.vector.tensor_scalar(
                            rs[:], ps_img(ct), 1.0 / WSC, beff, ALU.mult, ALU.add
                        )
                        nc.vector.tensor_scalar_max(rl[:], rs[:], 0.0)
                    nc.vector.scalar_tensor_tensor(
                        dst[:, y0 : y0 + RPC, :], e[:], 1.0, rl[:], ALU.min, ALU.add,
                        accum_out=ks24[:, r : r + 1] if acc else None,
                    )
                nc.vector.tensor_scalar(
                    vh[:, y0 : y0 + RPC, :], ps_img(cv), 1.0 / WSC, bev,
                    ALU.mult, ALU.add,
                )
                nc.scalar.activation(
                    lep[:, y0 : y0 + RPC, :], ps_img(cl), AF.Identity,
                    bias=blep, scale=1.0 / WSC,
                )

        # ---- k_mean from the per-chunk accum_out sums -> replicated lhsT
        nc.vector.tensor_reduce(ks_sb[:], ks24[:], axis=mybir.AxisListType.X, op=ALU.add)
        nc.vector.tensor_scalar(
            kmr_sb[:], ones_sb[:], ks_sb[:], ZS, ALU.mult, ALU.mult
        )

        # ---- kv = s2 * k~^T v~ via PE transposes, 72 token chunks;
        # kt/vt share one PSUM bank, one evac copy alternating DVE/Act
        kv_sb = kvs.tile([C, C], BF16, tag="kv")
        with tc.tile_pool(name="kvp", bufs=1, space="PSUM") as kvp, \
             tc.tile_pool(name="tpp", bufs=6, space="PSUM") as tpp:
          kvacc = kvp.tile([C, C], F32, tag="kvacc")
          kf = kh[:].rearrange("p a b -> p (a b)")
          vf = vh[:].rearrange("p a b -> p (a b)")
          for j in range(HW // C):
            tp = tpp.tile([C, 2 * C], BF16, tag="tp")
            nc.tensor.transpose(tp[:, :C], kf[:, j * C : (j + 1) * C], id_sb[:])
            nc.tensor.transpose(tp[:, C:], vf[:, j * C : (j + 1) * C], id_sb[:])
            kvt = ttmp.tile([C, 2 * C], BF16, tag="kvt")
            if j % 2 == 0:
                nc.vector.tensor_copy(kvt[:], tp[:])
            else:
                nc.scalar.activation(kvt[:], tp[:], AF.Copy)
            nc.tensor.matmul(
                kvacc[:], kvt[:, :C], kvt[:, C:],
                start=(j == 0), stop=(j == HW // C - 1),
            )
          nc.scalar.activation(kv_sb[:], kvacc[:], AF.Copy, scale=float(S2))

        # ---- res = (q~ @ kv)(1 + 1/z) - z*vbar ; + lepe ; * o ; proj ; out
        with tc.tile_pool(name="ps_e", bufs=2, space="PSUM") as ps, \
             tc.tile_pool(name="ps_f", bufs=2, space="PSUM") as psf:
          for r in range(NCH + 2):
            if r < NCH:
                y0 = r * RPC
                rq = qh[:, y0 : y0 + RPC, :]
                rv = vh[:, y0 : y0 + RPC, :]
                zp = ps.tile([C, RPC, W], F32, tag="zp")
                nc.tensor.matmul(zp[:], kmr_sb[:], rq, start=True, stop=True)
                vb = ps.tile([C, RPC, W], F32, tag="vb")
                nc.tensor.matmul(vb[:], inv_sb[:], rv, start=True, stop=True)
                rp = ps.tile([C, RPC, W], F32, tag="rp")
                nc.tensor.matmul(rp[:], kv_sb[:], rq, start=True, stop=True)
            if r >= 2:
                yp = (r - 2) * RPC
                pp = psf.tile([C, RPC, W], F32, tag="pp")
                nc.tensor.matmul(
                    pp[:], wp_sb[:], res[:, yp : yp + RPC, :], start=True, stop=True
                )
                og = ostg.tile([C, RPC, W], BF16, tag="og")
                nc.scalar.activation(og[:], pp[:], AF.Identity, bias=bprj)
                nc.sync.dma_start(out=out_d[b, :, yp : yp + RPC, :], in_=og[:])
            if r < NCH:
                rr_t = ctmp.tile([C, RPC, W], BF16, tag="rr")
                nc.vector.reciprocal(rr_t[:], zp[:])
                t1 = ctmp.tile([C, RPC, W], BF16, tag="t1")
                nc.vector.scalar_tensor_tensor(
                    t1[:], rr_t[:], 1.0, rp[:], ALU.add, ALU.mult
                )
                vbs = ctmp.tile([C, RPC, W], BF16, tag="vbs")
                nc.scalar.activation(vbs[:], vb[:], AF.Copy)
                t2 = ctmp.tile([C, RPC, W], BF16, tag="t2")
                nc.vector.tensor_tensor(t2[:], zp[:], vbs[:], ALU.mult)
                t3 = ctmp.tile([C, RPC, W], BF16, tag="t3")
                nc.gpsimd.tensor_tensor(t3[:], t1[:], t2[:], ALU.subtract)
                t4 = ctmp.tile([C, RPC, W], BF16, tag="t4")
                nc.gpsimd.tensor_tensor(t4[:], t3[:], lep[:, y0 : y0 + RPC, :], ALU.add)
                nc.vector.tensor_tensor(
                    res[:, y0 : y0 + RPC, :], t4[:], o_sb[:, y0 : y0 + RPC, :], ALU.mult
                )


_CACHE = {}


def _get_nc(zb: bool = True):
    key = ("nc", zb)
    if key not in _CACHE:
        nc = bacc.Bacc("TRN2", target_bir_lowering=False, debug=False)
        with tile.TileContext(nc, pool_alloc_mode="queue") as tc, \
             nc.allow_low_precision(reason="bf16/fp8 compute pipeline"):
            _build(tc, zb)
        nc.compile()
        _CACHE[key] = nc
    return _CACHE[key]


def _diag(v):
    d = np.zeros((C, C), np.float32)
    np.fill_diagonal(d, v)
    return d


def kernel(**inputs) -> np.ndarray:
    x = np.asarray(inputs["x"], np.float32)
    w_qkvo = np.asarray(inputs["w_qkvo"], np.float32)[:, :, 0, 0]  # (4C, C)
    b_qkvo = np.asarray(inputs["b_qkvo"], np.float32)
    w_lepe = np.asarray(inputs["w_lepe"], np.float32)[:, 0]        # (C,5,5)
    b_lepe = np.asarray(inputs["b_lepe"], np.float32)
    w_proj = np.asarray(inputs["w_proj"], np.float32)[:, :, 0, 0]
    b_proj = np.asarray(inputs["b_proj"], np.float32)
    w_q = np.asarray(inputs["w_q"], np.float32)[:, 0]              # (C,3,3)
    b_q = np.asarray(inputs["b_q"], np.float32)
    w_k = np.asarray(inputs["w_k"], np.float32)[:, 0]
    b_k = np.asarray(inputs["b_k"], np.float32)
    w_v = np.asarray(inputs["w_v"], np.float32)[:, 0]
    b_v = np.asarray(inputs["b_v"], np.float32)

    fp8 = ml_dtypes.float8_e4m3
    bf16 = ml_dtypes.bfloat16

    # DoubleRow pair diagonals (w * WSC); taps without a vertical partner
    # get a zero second diagonal
    pairs = np.zeros((NPR, 2, C, C), np.float32)
    for t, wt in enumerate((w_q, w_k, w_v)):
        for i, (dy, dx) in enumerate(P3):
            pairs[5 * t + i, 0] = _diag(wt[:, dy + 1, dx + 1] * WSC)
            if dy + 3 <= 2:
                pairs[5 * t + i, 1] = _diag(wt[:, dy + 3, dx + 1] * WSC)
    for i, (dy, dx) in enumerate(P5):
        pairs[15 + i, 0] = _diag(w_lepe[:, dy + 2, dx + 2] * WSC)
        if dy + 4 <= 4:
            pairs[15 + i, 1] = _diag(w_lepe[:, dy + 4, dx + 2] * WSC)
    dwpairs = np.ascontiguousarray(pairs.transpose(2, 0, 1, 3)).astype(fp8)

    # center+residual columns: Wg row-scaled by (1+w_center)*WSC, transposed
    wqkvoc = np.concatenate(
        [
            (w_qkvo[t * C : (t + 1) * C] * ((1.0 + wt[:, 1, 1]) * WSC)[:, None]).T
            for t, wt in enumerate((w_q, w_k, w_v))
        ],
        axis=1,
    )  # (C, 3C)

    beff_q = b_q + (1.0 + w_q[:, 1, 1]) * b_qkvo[:C]
    beff_k = b_k + (1.0 + w_k[:, 1, 1]) * b_qkvo[C : 2 * C]
    beff_v = b_v + (1.0 + w_v[:, 1, 1]) * b_qkvo[2 * C : 3 * C]
    biases = np.stack(
        [
            b_qkvo[:C], b_qkvo[C : 2 * C], b_qkvo[2 * C : 3 * C], b_qkvo[3 * C :],
            beff_q, beff_k, beff_v, b_lepe, b_proj,
        ],
        axis=1,
    ).astype(np.float32)
    zb = not (np.any(beff_q) or np.any(beff_k))

    shared = {
        "wqkvo_t": np.ascontiguousarray(w_qkvo.T),
        "wqkvoc_t": np.ascontiguousarray(wqkvoc),
        "wproj_t": np.ascontiguousarray(w_proj.T),
        "dwpairs": dwpairs,
        "biases": biases,
        "ident": np.eye(C, dtype=np.float32),
    }
    xb = x.astype(bf16).reshape(NCORES, BPC, C, H, W)
    in_maps = [{"x": np.ascontiguousarray(xb[i]), **shared} for i in range(NCORES)]

    nc = _get_nc(zb)
    _CACHE["last_in_maps"] = in_maps
    r = run_bass_kernel_spmd(
        nc, in_maps, core_ids=list(range(NCORES)),
        trace=bool(int(os.environ.get("KERNEL_TRACE", "0"))),
    )
    _CACHE["last_results"] = r
    out = np.stack([np.asarray(r.results[i]["out"]) for i in range(NCORES)])
    return out.astype(np.float32).reshape(B, C, H, W)


# revision 32
# speedup vs baseline: 1.1817x; 1.1640x over previous
"""AddLinearAttention Trainium2 kernel — 8-core data-parallel over batch.

B=16, C=128, H=W=96. Each of 8 cores handles 2 batches, channel-major
(C=128 partitions, HW free). Depthwise convs run as fp8 DoubleRow
pair-matmuls over zero-padded fp8 images (two vertically-adjacent taps
per PE instruction at 0.5 cyc/row); the center tap + residual ride a
row-scaled copy of the 1x1-conv weights written strided into the same
PSUM accumulation, so elu() reads conv+residual straight from PSUM.
"""

import os
from contextlib import ExitStack

import numpy as np
import ml_dtypes

import concourse.bass as bass
import concourse.tile as tile
from concourse import bacc, mybir
from concourse._compat import with_exitstack
from concourse.bass_utils import run_bass_kernel_spmd

B, C, H, W = 16, 128, 96, 96
HW = H * W
NCORES = 8
BPC = B // NCORES          # batches per core
RPC = 4                    # image rows per outer chunk
NCH = H // RPC             # chunks per image (24)
PAD = 2
WP = W + 2 * PAD           # 100
HP = H + 2 * PAD + 3       # 103: tail rows keep flat tap runs in-bounds
SCALE = C ** (-0.5)
S2 = SCALE / HW            # kv scale
ZS = SCALE / HW            # z scale folds k_mean's 1/HW and the C^-0.5
WSC = 256.0                # fp8 tap-weight scale (power of 2)

F32 = mybir.dt.float32
BF16 = mybir.dt.bfloat16
FP8 = mybir.dt.float8e4
AF = mybir.ActivationFunctionType
ALU = mybir.AluOpType
PM = mybir.MatmulPerfMode

DX3 = (-1, 0, 1)
DX5 = (-2, -1, 0, 1, 2)
# vertical DoubleRow pairs ((dy_a,dx),(dy_a+2,dx)); taps with no vertical
# partner get a zero second diagonal (center (0,0) rides wqkvoc instead)
P3 = [(-1, dx) for dx in DX3] + [(0, -1), (0, 1)]                  # 5
P5 = [(-2, dx) for dx in DX5] + [(-1, dx) for dx in DX5] + [(2, dx) for dx in DX5]  # 15
NPR = 3 * len(P3) + len(P5)                        # 30


@with_exitstack
def _build(ctx: ExitStack, tc: tile.TileContext, zb: bool):
    nc = tc.nc
    x_d = nc.dram_tensor("x", [BPC, C, H, W], BF16, kind="ExternalInput").ap()
    wqkvo_d = nc.dram_tensor("wqkvo_t", [C, 4 * C], F32, kind="ExternalInput").ap()
    wqkvoc_d = nc.dram_tensor("wqkvoc_t", [C, 3 * C], F32, kind="ExternalInput").ap()
    wproj_d = nc.dram_tensor("wproj_t", [C, C], F32, kind="ExternalInput").ap()
    pairs_d = nc.dram_tensor("dwpairs", [C, NPR, 2, C], FP8, kind="ExternalInput").ap()
    bias_d = nc.dram_tensor("biases", [C, 9], F32, kind="ExternalInput").ap()
    ident_d = nc.dram_tensor("ident", [C, C], F32, kind="ExternalInput").ap()
    out_d = nc.dram_tensor("out", [BPC, C, H, W], BF16, kind="ExternalOutput").ap()

    const = ctx.enter_context(tc.tile_pool(name="const", bufs=1))
    wq_sb = const.tile([C, 4 * C], BF16, tag="wq")
    wqc_sb = const.tile([C, 3 * C], BF16, tag="wqc")
    wp_sb = const.tile([C, C], BF16, tag="wp")
    pr_sb = const.tile([C, NPR, 2, C], FP8, tag="pr")
    bi_sb = const.tile([C, 9], F32, tag="bi")
    id_sb = const.tile([C, C], BF16, tag="id")
    ones_sb = const.tile([C, C], BF16, tag="ones")
    inv_sb = const.tile([C, C], BF16, tag="inv")
    kmr_sb = const.tile([C, C], BF16, tag="kmr")
    ks_sb = const.tile([C, 1], F32, tag="ks")

    # f32 -> bf16 casts ride the SWDGE DMA; fp8 ships raw
    nc.gpsimd.dma_start(out=wq_sb[:], in_=wqkvo_d[:])
    nc.gpsimd.dma_start(out=wqc_sb[:], in_=wqkvoc_d[:])
    nc.gpsimd.dma_start(out=wp_sb[:], in_=wproj_d[:])
    nc.sync.dma_start(out=pr_sb[:], in_=pairs_d[:])
    nc.sync.dma_start(out=bi_sb[:], in_=bias_d[:])
    nc.gpsimd.dma_start(out=id_sb[:], in_=ident_d[:])
    nc.vector.memset(ones_sb[:], 1.0)
    nc.vector.memset(inv_sb[:], 1.0 / C)

    bq, bk, bv, bo = (bi_sb[:, i : i + 1] for i in range(4))
    beq, bek, bev, blep, bprj = (bi_sb[:, i : i + 1] for i in range(4, 9))

    pads = ctx.enter_context(tc.tile_pool(name="pads", bufs=1))
    qpad = pads.tile([C, HP, WP], FP8, tag="qpad")
    kpad = pads.tile([C, HP, WP], FP8, tag="kpad")
    vpad = pads.tile([C, HP, WP], FP8, tag="vpad")
    for t in (qpad, kpad, vpad):
        # interior is rewritten every batch; zero only the border cells
        nc.vector.memset(t[:, :PAD, :], 0.0)                  # top rows
        nc.vector.memset(t[:, PAD + H :, :], 0.0)             # bottom + tail
        nc.vector.memset(t[:, PAD : PAD + H, :PAD], 0.0)      # left cols
        nc.vector.memset(t[:, PAD : PAD + H, PAD + W :], 0.0)  # right cols

    APc = type(qpad[:])

    def pad_pair(pad, dy_a, dx, y0, s):
        f = pad[:]
        off = (PAD + y0 + 2 * s + dy_a) * WP + (PAD + dx)
        return APc(f.tensor, f.offset + off, [[f.ap[0][0], C], [2 * WP, 2], [1, 2 * WP]])

    def pad_run(pad, dy, dx, y0, s):
        f = pad[:]
        off = (PAD + y0 + 2 * s + dy) * WP + (PAD + dx)
        return APc(f.tensor, f.offset + off, [[f.ap[0][0], C], [1, 2 * WP]])

    def ps_run(pt, s):
        f = pt[:]
        return APc(f.tensor, f.offset + s * 2 * WP, [[f.ap[0][0], C], [1, 2 * WP]])

    def ps_img(pt):
        f = pt[:]
        return APc(f.tensor, f.offset, [[f.ap[0][0], C], [WP, RPC], [1, W]])

    flats = ctx.enter_context(tc.tile_pool(name="flats", bufs=1))
    xin = ctx.enter_context(tc.tile_pool(name="xin", bufs=2))
    etmp = ctx.enter_context(tc.tile_pool(name="etmp", bufs=4))
    ctmp = ctx.enter_context(tc.tile_pool(name="ctmp", bufs=2))
    ttmp = ctx.enter_context(tc.tile_pool(name="ttmp", bufs=6))
    ostg = ctx.enter_context(tc.tile_pool(name="ostg", bufs=2))
    kvs = ctx.enter_context(tc.tile_pool(name="kvs", bufs=2))

    for b in range(BPC):
        x_sb = xin.tile([C, H, W], BF16, tag="x")
        o_sb = flats.tile([C, H, W], BF16, tag="o")
        qh = flats.tile([C, H, W], BF16, tag="qh")
        kh = flats.tile([C, H, W], BF16, tag="kh")
        vh = flats.tile([C, H, W], BF16, tag="vh")
        lep = flats.tile([C, H, W], BF16, tag="lep")
        res = flats.tile([C, H, W], BF16, tag="kh")    # kh dead after kv phase
        ks24 = flats.tile([C, NCH], F32, tag="ks24")   # per-chunk kh row sums
        nc.sync.dma_start(out=x_sb[:], in_=x_d[b])

        # ---- phase A: qkvo 1x1 (lead 2) + fp8 DoubleRow depthwise convs
        with tc.tile_pool(name="ps_a", bufs=2, space="PSUM") as psa, \
             tc.tile_pool(name="ps_qk", bufs=2, space="PSUM") as pqk, \
             tc.tile_pool(name="ps_vl", bufs=1, space="PSUM") as pvl:
          for rr in range(NCH + 2):
            # qkvo matmuls + evacs for chunk rr, interleaved with taps of
            # chunk rr-2 so PE never stalls on the rotating qkvo banks
            def qkvo_mm(g):
                y0n = rr * RPC
                p = psa.tile([C, RPC, W], F32, tag="pg")
                nc.tensor.matmul(
                    p[:], wq_sb[:, g * C : (g + 1) * C],
                    x_sb[:, y0n : y0n + RPC, :], start=True, stop=True,
                )
                return p

            def qkvo_evac(g, p):
                y0n = rr * RPC
                if g < 3:
                    pad = (qpad, kpad, vpad)[g]
                    dstw = pad[:, PAD + y0n : PAD + y0n + RPC, PAD : PAD + W]
                    bb = (bq, bk, bv)[g]
                    if g == 0:
                        nc.vector.tensor_scalar_add(dstw, p[:], bb)
                    else:
                        nc.scalar.activation(dstw, p[:], AF.Identity, bias=bb)
                else:
                    nc.scalar.activation(
                        o_sb[:, y0n : y0n + RPC, :], p[:], AF.Identity, bias=bo
                    )

            do_qkvo = rr < NCH
            do_taps = rr >= 2
            r = rr - 2
            y0 = r * RPC

            if do_qkvo:
                pq_ = qkvo_mm(0)
                pk_ = qkvo_mm(1)
                qkvo_evac(0, pq_)
                qkvo_evac(1, pk_)

            def tap_group(pool, tag, pr0, taps, pad, center_col):
                # PSUM bank rule: a second start=True closes the prior group,
                # so each sub-group OPENS with its own center half (start=True)
                # and the DoubleRow pairs accumulate behind it.
                ct = pool.tile([C, RPC, WP], F32, tag=tag)
                ctf = ct[:]
                for s in (0, 1):
                    if center_col is not None:
                        cout = APc(ctf.tensor, ctf.offset + s * 2 * WP,
                                   [[ctf.ap[0][0], C], [WP, 2], [1, W]])
                        nc.tensor.matmul(
                            cout, wqc_sb[:, center_col * C : (center_col + 1) * C],
                            x_sb[:, y0 + 2 * s : y0 + 2 * s + 2, :],
                            start=True, stop=False, skip_group_check=True,
                        )
                    for i, (dy, dx) in enumerate(taps):
                        nc.tensor.matmul(
                            ps_run(ct, s), pr_sb[:, pr0 + i],
                            pad_pair(pad, dy, dx, y0, s),
                            start=(center_col is None and i == 0),
                            stop=(s == 1 and i == len(taps) - 1),
                            perf_mode=PM.DoubleRow, skip_group_check=True,
                        )
                return ct

            cq = ck = cv = cl = None
            if do_taps:
                cq = tap_group(pqk, "cq", 0, P3, qpad, 0)
                ck = tap_group(pqk, "ck", 5, P3, kpad, 1)
            if do_qkvo:
                pv_ = qkvo_mm(2)
                po_ = qkvo_mm(3)
                qkvo_evac(2, pv_)
                qkvo_evac(3, po_)
            if do_taps:
                cv = tap_group(pvl, "cv", 10, P3, vpad, 2)
                cl = tap_group(pvl, "cl", 15, P5, vpad, None)

                # elu(s)+1 = min(exp(s),1) + max(s,0), s read straight from PSUM
                for ct, beff, dst, acc in ((cq, beq, qh, None),
                                           (ck, bek, kh, True)):
                    e = etmp.tile([C, RPC, W], BF16, tag="e")
                    nc.scalar.activation(
                        e[:], ps_img(ct), AF.Exp, bias=beff, scale=1.0 / WSC
                    )
                    rl = etmp.tile([C, RPC, W], BF16, tag="rl")
                    if zb:
                        nc.vector.tensor_scalar(
                            rl[:], ps_img(ct), 1.0 / WSC, 0.0, ALU.mult, ALU.max
                        )
                    else:
                        rs = etmp.tile([C, RPC, W], BF16, tag="rs")
                        nc.vector.tensor_scalar(
                            rs[:], ps_img(ct), 1.0 / WSC, beff, ALU.mult, ALU.add
                        )
                        nc.vector.tensor_scalar_max(rl[:], rs[:], 0.0)
                    nc.vector.scalar_tensor_tensor(
                        dst[:, y0 : y0 + RPC, :], e[:], 1.0, rl[:], ALU.min, ALU.add,
                        accum_out=ks24[:, r : r + 1] if acc else None,
                    )
                nc.vector.tensor_scalar(
                    vh[:, y0 : y0 + RPC, :], ps_img(cv), 1.0 / WSC, bev,
                    ALU.mult, ALU.add,
                )
                nc.scalar.activation(
                    lep[:, y0 : y0 + RPC, :], ps_img(cl), AF.Identity,
                    bias=blep, scale=1.0 / WSC,
                )

        # ---- k_mean from the per-chunk accum_out sums -> replicated lhsT
        nc.vector.tensor_reduce(ks_sb[:], ks24[:], axis=mybir.AxisListType.X, op=ALU.add)
        nc.vector.tensor_scalar(
            kmr_sb[:], ones_sb[:], ks_sb[:], ZS, ALU.mult, ALU.mult
        )

        # ---- kv = s2 * k~^T v~ via PE transposes, 72 token chunks;
        # kt/vt share one PSUM bank, one evac copy alternating DVE/Act
        kv_sb = kvs.tile([C, C], BF16, tag="kv")
        with tc.tile_pool(name="kvp", bufs=1, space="PSUM") as kvp, \
             tc.tile_pool(name="tpp", bufs=6, space="PSUM") as tpp:
          kvacc = kvp.tile([C, C], F32, tag="kvacc")
          kf = kh[:].rearrange("p a b -> p (a b)")
          vf = vh[:].rearrange("p a b -> p (a b)")
          for j in range(HW // C):
            tp = tpp.tile([C, 2 * C], BF16, tag="tp")
            nc.tensor.transpose(tp[:, :C], kf[:, j * C : (j + 1) * C], id_sb[:])
            nc.tensor.transpose(tp[:, C:], vf[:, j * C : (j + 1) * C], id_sb[:])
            kvt = ttmp.tile([C, 2 * C], BF16, tag="kvt")
            if j % 2 == 0:
                nc.vector.tensor_copy(kvt[:], tp[:])
            else:
                nc.scalar.activation(kvt[:], tp[:], AF.Copy)
            nc.tensor.matmul(
                kvacc[:], kvt[:, :C], kvt[:, C:],
                start=(j == 0), stop=(j == HW // C - 1),
            )
          nc.scalar.activation(kv_sb[:], kvacc[:], AF.Copy, scale=float(S2))

        # ---- res = (q~ @ kv)(1 + 1/z) - z*vbar ; + lepe ; * o ; proj ; out
        with tc.tile_pool(name="ps_e", bufs=2, space="PSUM") as ps, \
             tc.tile_pool(name="ps_f", bufs=2, space="PSUM") as psf:
          for r in range(NCH + 2):
            if r < NCH:
                y0 = r * RPC
                rq = qh[:, y0 : y0 + RPC, :]
                rv = vh[:, y0 : y0 + RPC, :]
                zp = ps.tile([C, RPC, W], F32, tag="zp")
                nc.tensor.matmul(zp[:], kmr_sb[:], rq, start=True, stop=True)
                vb = ps.tile([C, RPC, W], F32, tag="vb")
                nc.tensor.matmul(vb[:], inv_sb[:], rv, start=True, stop=True)
                rp = ps.tile([C, RPC, W], F32, tag="rp")
                nc.tensor.matmul(rp[:], kv_sb[:], rq, start=True, stop=True)
            if r >= 2:
                yp = (r - 2) * RPC
                pp = psf.tile([C, RPC, W], F32, tag="pp")
                nc.tensor.matmul(
                    pp[:], wp_sb[:], res[:, yp : yp + RPC, :], start=True, stop=True
                )
                og = ostg.tile([C, RPC, W], BF16, tag="og")
                nc.scalar.activation(og[:], pp[:], AF.Identity, bias=bprj)
                nc.sync.dma_start(out=out_d[b, :, yp : yp + RPC, :], in_=og[:])
            if r < NCH:
                zps = ctmp.tile([C, RPC, W], BF16, tag="zps")
                nc.scalar.activation(zps[:], zp[:], AF.Copy)
                rr_t = ctmp.tile([C, RPC, W], BF16, tag="rr")
                nc.vector.reciprocal(rr_t[:], zps[:])
                t1 = ctmp.tile([C, RPC, W], BF16, tag="t1")
                nc.vector.scalar_tensor_tensor(
                    t1[:], rr_t[:], 1.0, rp[:], ALU.add, ALU.mult
                )
                vbs = ctmp.tile([C, RPC, W], BF16, tag="vbs")
                nc.scalar.activation(vbs[:], vb[:], AF.Copy)
                t2 = ctmp.tile([C, RPC, W], BF16, tag="t2")
                nc.vector.tensor_tensor(t2[:], zps[:], vbs[:], ALU.mult)
                t3 = ctmp.tile([C, RPC, W], BF16, tag="t3")
                nc.gpsimd.tensor_tensor(t3[:], t1[:], t2[:], ALU.subtract)
                t4 = ctmp.tile([C, RPC, W], BF16, tag="t4")
                nc.gpsimd.tensor_tensor(t4[:], t3[:], lep[:, y0 : y0 + RPC, :], ALU.add)
                nc.vector.tensor_tensor(
                    res[:, y0 : y0 + RPC, :], t4[:], o_sb[:, y0 : y0 + RPC, :], ALU.mult
                )


_CACHE = {}


def _get_nc(zb: bool = True):
    key = ("nc", zb)
    if key not in _CACHE:
        nc = bacc.Bacc("TRN2", target_bir_lowering=False, debug=False)
        with tile.TileContext(nc, pool_alloc_mode="queue") as tc, \
             nc.allow_low_precision(reason="bf16/fp8 compute pipeline"):
            _build(tc, zb)
        nc.compile()
        _CACHE[key] = nc
    return _CACHE[key]


def _diag(v):
    d = np.zeros((C, C), np.float32)
    np.fill_diagonal(d, v)
    return d


def kernel(**inputs) -> np.ndarray:
    x = np.asarray(inputs["x"], np.float32)
    w_qkvo = np.asarray(inputs["w_qkvo"], np.float32)[:, :, 0, 0]  # (4C, C)
    b_qkvo = np.asarray(inputs["b_qkvo"], np.float32)
    w_lepe = np.asarray(inputs["w_lepe"], np.float32)[:, 0]        # (C,5,5)
    b_lepe = np.asarray(inputs["b_lepe"], np.float32)
    w_proj = np.asarray(inputs["w_proj"], np.float32)[:, :, 0, 0]
    b_proj = np.asarray(inputs["b_proj"], np.float32)
    w_q = np.asarray(inputs["w_q"], np.float32)[:, 0]              # (C,3,3)
    b_q = np.asarray(inputs["b_q"], np.float32)
    w_k = np.asarray(inputs["w_k"], np.float32)[:, 0]
    b_k = np.asarray(inputs["b_k"], np.float32)
    w_v = np.asarray(inputs["w_v"], np.float32)[:, 0]
    b_v = np.asarray(inputs["b_v"], np.float32)

    fp8 = ml_dtypes.float8_e4m3
    bf16 = ml_dtypes.bfloat16

    # DoubleRow pair diagonals (w * WSC); taps without a vertical partner
    # get a zero second diagonal
    pairs = np.zeros((NPR, 2, C, C), np.float32)
    for t, wt in enumerate((w_q, w_k, w_v)):
        for i, (dy, dx) in enumerate(P3):
            pairs[5 * t + i, 0] = _diag(wt[:, dy + 1, dx + 1] * WSC)
            if dy + 3 <= 2:
                pairs[5 * t + i, 1] = _diag(wt[:, dy + 3, dx + 1] * WSC)
    for i, (dy, dx) in enumerate(P5):
        pairs[15 + i, 0] = _diag(w_lepe[:, dy + 2, dx + 2] * WSC)
        if dy + 4 <= 4:
            pairs[15 + i, 1] = _diag(w_lepe[:, dy + 4, dx + 2] * WSC)
    dwpairs = np.ascontiguousarray(pairs.transpose(2, 0, 1, 3)).astype(fp8)

    # center+residual columns: Wg row-scaled by (1+w_center)*WSC, transposed
    wqkvoc = np.concatenate(
        [
            (w_qkvo[t * C : (t + 1) * C] * ((1.0 + wt[:, 1, 1]) * WSC)[:, None]).T
            for t, wt in enumerate((w_q, w_k, w_v))
        ],
        axis=1,
    )  # (C, 3C)

    beff_q = b_q + (1.0 + w_q[:, 1, 1]) * b_qkvo[:C]
    beff_k = b_k + (1.0 + w_k[:, 1, 1]) * b_qkvo[C : 2 * C]
    beff_v = b_v + (1.0 + w_v[:, 1, 1]) * b_qkvo[2 * C : 3 * C]
    biases = np.stack(
        [
            b_qkvo[:C], b_qkvo[C : 2 * C], b_qkvo[2 * C : 3 * C], b_qkvo[3 * C :],
            beff_q, beff_k, beff_v, b_lepe, b_proj,
        ],
        axis=1,
    ).astype(np.float32)
    zb = not (np.any(beff_q) or np.any(beff_k))

    shared = {
        "wqkvo_t": np.ascontiguousarray(w_qkvo.T),
        "wqkvoc_t": np.ascontiguousarray(wqkvoc),
        "wproj_t": np.ascontiguousarray(w_proj.T),
        "dwpairs": dwpairs,
        "biases": biases,
        "ident": np.eye(C, dtype=np.float32),
    }
    xb = x.astype(bf16).reshape(NCORES, BPC, C, H, W)
    in_maps = [{"x": np.ascontiguousarray(xb[i]), **shared} for i in range(NCORES)]

    nc = _get_nc(zb)
    _CACHE["last_in_maps"] = in_maps
    r = run_bass_kernel_spmd(
        nc, in_maps, core_ids=list(range(NCORES)),
        trace=bool(int(os.environ.get("KERNEL_TRACE", "0"))),
    )
    _CACHE["last_results"] = r
    out = np.stack([np.asarray(r.results[i]["out"]) for i in range(NCORES)])
    return out.astype(np.float32).reshape(B, C, H, W)
